# revision 24
# baseline (speedup 1.0000x reference)
"""Trainium2 Bass kernel for nn_AutoformerBase (sparse_attention).

Algorithm (algebraically reduced from the reference):
  mean_value[b, tau] = (1/D) sum_d corr(q_proj_d, k_proj_d)[tau]
                     = sum_{t,j} rho[b,t,j] * k_in[b,(t-tau)%L,j],
    where rho = q_in @ A and A = (Wq @ Wk^T)/D   (bq/bk only shift
    mean_value by a per-batch constant -> no effect on top-k or softmax).
  index = top6 of mean over b of mean_value  (one tiny AllReduce)
  tmp_corr = softmax(mean_value[:, index])
  out = sum_k tmp_corr[:,k] * Z[:, (t+tau_k)%L, :] + (bv@Wo + bo),
    where Z = v_in @ (Wv @ Wo)  (rolls commute with the right-side matmul,
    softmax weights sum to 1 for the bias term).

Device strategy (per core, data-parallel over batch, 4 batches/core):
  - all matmuls in fp32r (1 cyc/row, ~1.3e-4 rel err, measured on HW)
  - mean_value via Gram diag-sums: G'[t',t] = sum_j kT[j,t'] rhoT[j,t],
    evacuated PSUM->SBUF through *sheared* write APs so that each column
    of the piece buffer holds one circulant diagonal; partition-reduce
    via gpsimd tensor_reduce(axis=C).
  - roll-aggregate on DVE with fused scalar_tensor_tensor MACs reading
    dynamic ds() windows of a doubled Z (exact fp32).
"""
import math
from contextlib import ExitStack
import numpy as np

import concourse.bass as bass
import concourse.mybir as mybir
import concourse.tile as tile
from concourse import bacc
from concourse.bass import ds
from concourse.tile import TileContext
from concourse.bass_utils import run_bass_kernel_spmd

B, L, D = 32, 1024, 512
NCORES = 8
BLOC = B // NCORES          # 4 batches per core
TOPK = 6
F32 = mybir.dt.float32
F32R = mybir.dt.float32r
U32 = mybir.dt.uint32
ALU = mybir.AluOpType
AFT = mybir.ActivationFunctionType

_CACHE = {}


def _build():
    nc = bacc.Bacc("TRN2", target_bir_lowering=False)

    qT_d = nc.dram_tensor("qT", [BLOC, D, L], F32R, kind="ExternalInput")
    kT_d = nc.dram_tensor("kT", [BLOC, D, L], F32R, kind="ExternalInput")
    vT_d = nc.dram_tensor("vT", [BLOC, D, L], F32R, kind="ExternalInput")
    A_d = nc.dram_tensor("A", [D, D], F32R, kind="ExternalInput")
    I_d = nc.dram_tensor("I128", [128, 128], F32R, kind="ExternalInput")
    z0_d = nc.dram_tensor("z1152", [1152], F32R, kind="ExternalInput")
    Wc_d = nc.dram_tensor("Wc", [D, D], F32R, kind="ExternalInput")
    outT_d = nc.dram_tensor("outT", [BLOC, D, L], F32, kind="ExternalOutput")
    mvout_d = nc.dram_tensor("mvout", [BLOC, L], F32, kind="ExternalOutput")
    cc_in = nc.dram_tensor("cc_in", [1, L], F32)
    cc_out = nc.dram_tensor("cc_out", [1, L], F32, addr_space="Shared")

    NJC = D // 128           # 4 chunks of output-feature rows
    NIC = D // 128           # 4 chunks of contraction
    NTC = L // 512           # 2 free-dim chunks of 512
    NTR = L // 128           # 8 row-blocks of t'

    with TileContext(nc) as tc, ExitStack() as ctx:
        consts = ctx.enter_context(tc.tile_pool(name="consts", bufs=1))
        in_pool = ctx.enter_context(tc.tile_pool(name="in", bufs=2))
        rho_pool = ctx.enter_context(tc.tile_pool(name="rho", bufs=2))
        pc_pool = ctx.enter_context(tc.tile_pool(name="pc", bufs=1))
        v_pool = ctx.enter_context(tc.tile_pool(name="V", bufs=2))
        z_pool = ctx.enter_context(tc.tile_pool(name="z", bufs=2))
        out_pool = ctx.enter_context(tc.tile_pool(name="out", bufs=3))
        small = ctx.enter_context(tc.tile_pool(name="small", bufs=1))
        gst_pool = ctx.enter_context(tc.tile_pool(name="gst", bufs=2))
        ps_s = ctx.enter_context(tc.tile_pool(name="ps_s", bufs=2, space="PSUM"))
        ps_g = ctx.enter_context(tc.tile_pool(name="ps_g", bufs=2, space="PSUM"))
        ps_z = ps_s
        ps_v = ctx.enter_context(tc.tile_pool(name="ps_v", bufs=1, space="PSUM"))
        ps_w = ctx.enter_context(tc.tile_pool(name="ps_w", bufs=1, space="PSUM"))

        # ---- constants ----
        A_sb = consts.tile([128, NIC, D], F32R)
        Wc_sb = consts.tile([128, NIC, D], F32R)
        nc.sync.dma_start(out=A_sb, in_=A_d.rearrange("(ic p) j -> p ic j", p=128))
        nc.sync.dma_start(out=Wc_sb, in_=Wc_d.rearrange("(ic p) j -> p ic j", p=128))
        ones1 = consts.tile([1, 128], F32)
        nc.vector.memset(ones1, 1.0)
        ones128 = consts.tile([128, 1], F32)
        nc.vector.memset(ones128, 1.0)
        onesq = consts.tile([128, 128], F32)
        nc.vector.memset(onesq, 1.0)
        I_sb = consts.tile([128, 128], F32R)
        nc.sync.dma_start(out=I_sb, in_=I_d.ap())

        # persistent sheared piece buffers: borders stay zero across reuse
        NPB = 2
        pbs = []
        for i in range(NPB):
            pb_t = pc_pool.tile([128, 1152], F32R, tag=f"pb{i}", name=f"pb{i}")
            pbs.append(pb_t)
        for pb in pbs:
            zbc = bass.AP(tensor=z0_d, offset=0, ap=[[0, 128], [1, 1152]])
            nc.sync.dma_start(out=pb, in_=zbc)
        pb_i = 0

        bs = small.tile([1, L], F32)
        z_tiles = {}

        # ================= stats phase (per batch) =================
        def emit_stats(b):
            nonlocal pb_i
            qT = in_pool.tile([128, NIC, L], F32R, tag="xT", name=f"qT{b}")
            for ic in range(NIC):
                nc.sync.dma_start(
                    out=qT[:, ic], in_=qT_d.ap()[b, ic * 128:(ic + 1) * 128])
            kT = in_pool.tile([128, NIC, L], F32R, tag="xT", name=f"kT{b}")
            for ic in range(NIC):
                nc.sync.dma_start(
                    out=kT[:, ic], in_=kT_d.ap()[b, ic * 128:(ic + 1) * 128])

            # rhoT[j, t] = sum_i A[i, j] * qT[i, t]
            rho = rho_pool.tile([128, NJC, L], F32R, tag="rho", name=f"rho{b}")
            for jc in range(NJC):
                for tcc in range(NTC):
                    ps = ps_s.tile([128, 512], F32)
                    for ic in range(NIC):
                        nc.tensor.matmul(
                            ps,
                            lhsT=A_sb[:, ic, jc * 128:(jc + 1) * 128],
                            rhs=qT[:, ic, tcc * 512:(tcc + 1) * 512],
                            start=(ic == 0), stop=(ic == NIC - 1))
                    nc.scalar.copy(rho[:, jc, tcc * 512:(tcc + 1) * 512], ps)

            # Gram diag-sums: fold sheared pieces into V via PSUM-accum
            # matmuls with identity stationary (row-passthrough).
            vps = []
            for h in range(2):
                vps_t = ps_v.tile([128, 512], F32, tag=f"vps{h}", name=f"vps{h}_{b}")
                vps.append(vps_t)
            vps_started = [False, False]
            for tr in range(NTR):
                gst = gst_pool.tile([128, L], F32R, tag="gst")
                for tcc in range(NTC):
                    ps = ps_g.tile([128, 512], F32)
                    for jc in range(NJC):
                        nc.tensor.matmul(
                            ps,
                            lhsT=kT[:, jc, tr * 128:(tr + 1) * 128],
                            rhs=rho[:, jc, tcc * 512:(tcc + 1) * 512],
                            start=(jc == 0), stop=(jc == NJC - 1))
                    if tcc % 2 == 0:
                        nc.scalar.copy(gst[:, tcc * 512:(tcc + 1) * 512], ps)
                    else:
                        nc.vector.tensor_copy(gst[:, tcc * 512:(tcc + 1) * 512], ps)
                # sheared spill: pb[p, 127 - p + c] = gst[p, c]
                pb = pbs[pb_i % NPB]
                pb_i += 1
                shear = bass.AP(tensor=pb.tensor, offset=pb.offset + 128,
                                ap=[[1152 - 1, 128], [1, L]])
                nc.sync.dma_start(out=shear, in_=gst)
                # fold piece into V psum: tau = (base + kappa) % L
                base = (-128 - 128 * tr) % L
                segs = []
                k0 = 0
                while k0 < 1152:
                    t0 = (base + k0) % L
                    w = min(1152 - k0, L - t0)
                    # split further at the 512 psum-bank boundary of tau
                    while w > 0:
                        h = t0 // 512
                        w2 = min(w, (h + 1) * 512 - t0)
                        segs.append((k0, t0, w2, h))
                        k0 += w2
                        t0 += w2
                        w -= w2
                        if t0 == L:
                            t0 = 0
                for (k0, t0, w, h) in segs:
                    is_last = (tr == NTR - 1) and (k0 + w == 1152)
                    nc.tensor.matmul(
                        vps[h][:, t0 - h * 512:t0 - h * 512 + w],
                        lhsT=I_sb,
                        rhs=pb[:, k0:k0 + w],
                        start=(not vps_started[h]), stop=is_last,
                        skip_group_check=True)
                    vps_started[h] = True

            # V psum -> sbuf, then exact fp32 partition-reduce via ones matmul
            V = v_pool.tile([128, L], F32)
            nc.scalar.copy(V[:, 0:512], vps[0])
            nc.vector.tensor_copy(V[:, 512:L], vps[1])
            mv = small.tile([1, L], F32, tag=f"mv{b}", name=f"mv{b}")
            for h in range(2):
                mps = ps_w.tile([1, 512], F32, tag="mps")
                nc.tensor.matmul(mps, lhsT=ones128,
                                 rhs=V[:, h * 512:(h + 1) * 512],
                                 start=True, stop=True)
                nc.scalar.copy(mv[:, h * 512:(h + 1) * 512], mps)
            nc.sync.dma_start(out=mvout_d.ap()[b:b + 1, :], in_=mv)
            if b == 0:
                nc.vector.tensor_copy(bs, mv)
            else:
                nc.vector.tensor_add(bs, bs, mv)

        # ================= Z + aggregation =================
        def emit_z(b):
            vT = in_pool.tile([128, NIC, L], F32R, tag="xT", name=f"vT{b}")
            nc.sync.dma_start(out=vT, in_=vT_d.ap()[b].rearrange(
                "(ic p) t -> p ic t", p=128))
            Z = z_pool.tile([128, NJC, 2 * L], F32R, tag="Z", name=f"Z{b}")
            for jc in range(NJC):
                for tcc in range(NTC):
                    ps = ps_s.tile([128, 512], F32, tag="ps", name=f"zps{b}_{jc}_{tcc}")
                    for ic in range(NIC):
                        nc.tensor.matmul(
                            ps,
                            lhsT=Wc_sb[:, ic, jc * 128:(jc + 1) * 128],
                            rhs=vT[:, ic, tcc * 512:(tcc + 1) * 512],
                            start=(ic == 0), stop=(ic == NIC - 1))
                    nc.scalar.copy(Z[:, jc, tcc * 512:(tcc + 1) * 512], ps)
                    nc.gpsimd.tensor_copy(
                        Z[:, jc, L + tcc * 512:L + (tcc + 1) * 512],
                        Z[:, jc, tcc * 512:(tcc + 1) * 512])
            return Z

        def emit_agg_dve(b, Z):
            Zf = Z.bitcast(F32)
            for jc in range(NJC):
                for tcc in range(NTC):
                    acc = out_pool.tile([128, 512], F32, tag="acc", name=f"acc{b}_{jc}_{tcc}")
                    nc.vector.tensor_scalar(
                        out=acc,
                        in0=Zf[:, jc, ds(tau_v[0][tcc], 512)],
                        scalar1=w_bc[:, b, 0:1], scalar2=None, op0=ALU.mult)
                    for k in range(1, TOPK):
                        nc.vector.scalar_tensor_tensor(
                            out=acc,
                            in0=Zf[:, jc, ds(tau_v[k][tcc], 512)],
                            scalar=w_bc[:, b, k:k + 1],
                            in1=acc, op0=ALU.mult, op1=ALU.add)
                    nc.sync.dma_start(
                        out=outT_d.ap()[b, jc * 128:(jc + 1) * 128,
                                        tcc * 512:(tcc + 1) * 512],
                        in_=acc)

        def emit_agg_pe(b, Z, wI):
            for jc in range(NJC):
                for tcc in range(NTC):
                    ps = ps_g.tile([128, 512], F32, tag="ps",
                                   name=f"aps{b}_{jc}_{tcc}")
                    for k in range(TOPK):
                        nc.tensor.matmul(
                            ps,
                            lhsT=wI[k],
                            rhs=Z[:, jc, ds(tau_t[k][tcc], 512)],
                            start=(k == 0), stop=(k == TOPK - 1))
                    acc = out_pool.tile([128, 512], F32, tag="acc", name=f"pacc{b}_{jc}_{tcc}")
                    nc.scalar.copy(acc, ps)
                    nc.sync.dma_start(
                        out=outT_d.ap()[b, jc * 128:(jc + 1) * 128,
                                        tcc * 512:(tcc + 1) * 512],
                        in_=acc)

        emit_stats(0)
        emit_stats(1)
        emit_stats(2)
        emit_stats(3)
        z_tiles[0] = emit_z(0)
        z_tiles[1] = emit_z(1)

        # ================= collective + top-k =================
        nc.sync.dma_start(out=cc_in.ap(), in_=bs)
        nc.gpsimd.collective_compute(
            "AllReduce", ALU.add,
            replica_groups=[list(range(NCORES))],
            ins=[cc_in.ap()], outs=[cc_out.ap()])
        bm = small.tile([1, L], F32)
        nc.sync.dma_start(out=bm, in_=cc_out.ap())

        vals8 = small.tile([1, 8], F32)
        idx8 = small.tile([1, 8], U32)
        nc.vector.max_with_indices(vals8, idx8, bm)

        # tau_k registers on engines that need dynamic offsets
        tau_v = []
        for k in range(TOPK):
            r = nc.vector.alloc_register(f"tau{k}")
            nc.vector.reg_load(r, idx8[0:1, k:k + 1])
            base_sv = nc.snap(r, min_val=0, max_val=L - 1)
            r2 = nc.vector.alloc_register(f"tau{k}_hi")
            nc.vector.reg_load(r2, idx8[0:1, k:k + 1])
            nc.vector.reg_add(r2, r2, 512)
            hi_sv = nc.snap(r2, min_val=512, max_val=L - 1 + 512)
            tau_v.append((base_sv, hi_sv))
        tau_g = []
        for k in range(TOPK):
            r = nc.gpsimd.alloc_register(f"tau_g{k}")
            nc.gpsimd.reg_load(r, idx8[0:1, k:k + 1])
            tau_g.append(nc.snap(r, min_val=0, max_val=L - 1))
        tau_t = []
        for k in range(TOPK):
            r = nc.tensor.alloc_register(f"tau_t{k}")
            nc.tensor.reg_load(r, idx8[0:1, k:k + 1])
            base_sv = nc.snap(r, min_val=0, max_val=L - 1)
            r2 = nc.tensor.alloc_register(f"tau_t{k}_hi")
            nc.tensor.reg_load(r2, idx8[0:1, k:k + 1])
            nc.tensor.reg_add(r2, r2, 512)
            hi_sv = nc.snap(r2, min_val=512, max_val=L - 1 + 512)
            tau_t.append((base_sv, hi_sv))

        # ================= weights (batched over partitions 0-3) ==========
        w_bc = small.tile([128, BLOC, 8], F32)
        wraw = small.tile([1, BLOC, 8], F32)
        nc.vector.memset(wraw, 0.0)
        for b in range(BLOC):
            for k in range(TOPK):
                nc.gpsimd.dma_start(
                    out=wraw[0:1, b, k:k + 1],
                    in_=mvout_d.ap()[b:b + 1, ds(tau_g[k], 1)])
        w6 = wraw[:, :, 0:TOPK]
        mx = small.tile([1, BLOC], F32)
        nc.vector.tensor_reduce(out=mx, in_=w6, axis=mybir.AxisListType.X,
                                op=ALU.max)
        cen = small.tile([1, BLOC, 8], F32)
        mxb = bass.AP(tensor=mx.tensor, offset=mx.offset,
                      ap=[[BLOC, 1], [1, BLOC], [0, TOPK]])
        nc.vector.tensor_tensor(out=cen[:, :, 0:TOPK], in0=w6, in1=mxb,
                                op=ALU.subtract)
        ex = small.tile([1, BLOC, 8], F32)
        nc.vector.memset(ex, 0.0)
        nc.scalar.activation(ex[:, :, 0:TOPK], cen[:, :, 0:TOPK], AFT.Exp)
        sm = small.tile([1, BLOC], F32)
        nc.vector.tensor_reduce(out=sm, in_=ex[:, :, 0:TOPK],
                                axis=mybir.AxisListType.X, op=ALU.add)
        rc = small.tile([1, BLOC], F32)
        nc.vector.reciprocal(rc, sm)
        wn = small.tile([1, BLOC, 8], F32)
        rcb = bass.AP(tensor=rc.tensor, offset=rc.offset,
                      ap=[[BLOC, 1], [1, BLOC], [0, 8]])
        nc.vector.tensor_tensor(out=wn, in0=ex, in1=rcb, op=ALU.mult)
        psw = ps_w.tile([128, BLOC, 8], F32, tag="psw")
        nc.tensor.matmul(psw.rearrange("p a b -> p (a b)"), lhsT=ones1,
                         rhs=wn.rearrange("p a b -> p (a b)"),
                         start=True, stop=True)
        nc.scalar.copy(w_bc, psw)

        NPE = 2   # batches aggregated on the PE
        wIs = {}
        for b in range(NPE):
            wI = []
            for k in range(TOPK):
                t = small.tile([128, 128], F32R, tag=f"wI{b}_{k}",
                               name=f"wI{b}_{k}")
                nc.vector.tensor_scalar(out=t, in0=I_sb.bitcast(F32),
                                        scalar1=w_bc[:, b, k:k + 1],
                                        scalar2=None, op0=ALU.mult)
                wI.append(t)
            wIs[b] = wI

        emit_agg_pe(0, z_tiles[0], wIs[0])
        z_tiles[2] = emit_z(2)
        emit_agg_pe(1, z_tiles[1], wIs[1])
        z_tiles[3] = emit_z(3)
        emit_agg_dve(2, z_tiles[2])
        emit_agg_dve(3, z_tiles[3])

    nc.compile()
    return nc


def _get_nc():
    if "nc" not in _CACHE:
        _CACHE["nc"] = _build()
    return _CACHE["nc"]


def kernel(q_in, k_in, v_in, Wq, bq, Wk, bk, Wv, bv, Wo, bo):
    q_in = np.ascontiguousarray(q_in, dtype=np.float32)
    k_in = np.ascontiguousarray(k_in, dtype=np.float32)
    v_in = np.ascontiguousarray(v_in, dtype=np.float32)

    A = ((Wq.astype(np.float64) @ Wk.astype(np.float64).T) / D).astype(np.float32)
    Wc = (Wv.astype(np.float64) @ Wo.astype(np.float64)).astype(np.float32)
    c_row = (bv.astype(np.float64) @ Wo.astype(np.float64) + bo).astype(np.float32)

    qT = np.ascontiguousarray(q_in.transpose(0, 2, 1))
    kT = np.ascontiguousarray(k_in.transpose(0, 2, 1))
    vT = np.ascontiguousarray(v_in.transpose(0, 2, 1))

    nc = _get_nc()
    in_maps = []
    for c in range(NCORES):
        sl = slice(c * BLOC, (c + 1) * BLOC)
        in_maps.append({
            "qT": qT[sl], "kT": kT[sl], "vT": vT[sl],
            "A": A, "Wc": Wc,
            "I128": np.eye(128, dtype=np.float32),
            "z1152": np.zeros(1152, dtype=np.float32),
        })
    res = run_bass_kernel_spmd(nc, in_maps, list(range(NCORES)))
    outT = np.concatenate([r["outT"] for r in res.results], axis=0)  # (B, D, L)
    out = outT.transpose(0, 2, 1) + c_row[None, None, :]
    return np.ascontiguousarray(out, dtype=np.float32)
